# revision 17
# baseline (speedup 1.0000x reference)
"""Deformable-conv (DefEDNet block) Trainium2 kernel, transfer-optimized.

The axon tunnel moves ~50MB/s, so the warm-call wall clock is dominated by
host<->device bytes.  Per core we upload a single bf16 blob (~2.4MB): the
padded image in [row, ch] layout plus the two weight products.  Everything
else is derived on device:
  - conv input: one XBAR transpose DMA into a row-pair layout, then a DVE
    lerp by rh (= partition_id & 1) selects this core's 66-row window.
  - gather table: tabL[r] = [flat row r | flat row r+1] (256B rows), built
    by two DRAM->DRAM DMAs; each old 512B-token gather becomes two 256B
    gathers (x-low pair at idx i, x-high pair at idx i+130).
  - index grids (pnxt/byt/...) are NEFF-baked consts; bxs comes from the
    partition id.
Output is bf16 (cast to f32 on host).  A cached jit runner avoids per-call
retracing, and the donated zero output buffers are created on device.
"""
import os
import numpy as np
import ml_dtypes

BF16 = ml_dtypes.bfloat16

B, C, H, W = 4, 64, 128, 128
Hp = Wp = 130
HALF = 64              # output rows per core
NQ = HALF * W          # queries per core (8192)
KC = 64                # column-pairs
TW = 512               # queries*pts per gather unit (128 qp x 4 ksub)
NR = 16                # r-units (KC / 4)
CONVROWS = 66
F23 = float(2 ** 23)

NXPT = 17160                    # 130*130 image rows + zero tail
XPT_E = NXPT * 64               # 1098240
FWL_E = 128 * 704               # 90112
PW2_E = 64 * 162                # 10368
OFF_XPT = 0
OFF_FWL = OFF_XPT + XPT_E
OFF_PW2 = OFF_FWL + FWL_E
TOTAL = OFF_PW2 + PW2_E         # 1198720
NTABL = 17152                   # pair-table rows, 128*134 (idx max 17029)

_cache = {}


def _build_program():
    import concourse.bass as bass
    import concourse.bacc as bacc
    import concourse.mybir as mybir
    import concourse.tile as tile

    dt = mybir.dt
    Alu = mybir.AluOpType
    nc = bacc.Bacc(num_swdge_queues=1)

    blob_d = nc.dram_tensor("blob", [TOTAL], dt.bfloat16, kind="ExternalInput")
    out_d = nc.dram_tensor("out", [C, HALF, W], dt.bfloat16, kind="ExternalOutput")

    offs_d = nc.dram_tensor("offs_scr", [18 * NQ], dt.float32, kind="Internal")
    sidx_d = nc.dram_tensor("sidx_scr", [NQ * 9 // 8 * 8], dt.int16, kind="Internal")
    u4_d = nc.dram_tensor("u4_scr", [4 * NQ * 9], dt.bfloat16, kind="Internal")
    tabl_d = nc.dram_tensor("tabl_scr", [NTABL, 128], dt.bfloat16, kind="Internal")

    # compile-time constant grids
    r_ = np.arange(-1, 2, dtype=np.float32)
    pnx, pny = np.meshgrid(r_, r_, indexing="ij")
    pnx, pny = pnx.ravel(), pny.ravel()
    col = np.arange(576)
    n_of = col // 64
    k_of = col % 64
    pnxt_np = np.broadcast_to(pnx[n_of][None, :], (128, 576)).astype(np.float32).copy()
    byt_np = np.broadcast_to((2 * k_of + 1 + pny[n_of])[None, :], (128, 576)).astype(np.float32).copy()
    pp_ = np.arange(128)
    pm64_np = ((pp_ % 64) + 1.0).astype(np.float32).reshape(128, 1)
    bjs_np = (pp_ // 64).astype(np.float32).reshape(128, 1)
    pnxt_c = nc.inline_tensor(pnxt_np, name="pnxt_c")
    byt_c = nc.inline_tensor(byt_np, name="byt_c")
    pm64_c = nc.inline_tensor(pm64_np, name="pm64_c")
    bjs_c = nc.inline_tensor(bjs_np, name="bjs_c")

    xpt_v = blob_d[OFF_XPT:OFF_XPT + XPT_E].rearrange("(r c) -> r c", c=64)
    xpair_v = blob_d[OFF_XPT:OFF_XPT + 17024 * 64].rearrange("(r c) -> r c", c=128)
    fwl_dv = blob_d[OFF_FWL:OFF_FWL + FWL_E].rearrange("(p c) -> p c", c=704)
    pw2_dv = blob_d[OFF_PW2:OFF_PW2 + PW2_E].rearrange("(c k) -> c k", k=162)

    with tile.TileContext(nc) as tc:
        with (
            tc.tile_pool(name="persist", bufs=1) as pp,
            tc.tile_pool(name="wtmp", bufs=2) as wp,
            tc.tile_pool(name="wtmp1", bufs=1) as wp1,
            tc.tile_pool(name="gpool", bufs=8) as gp,
            tc.tile_pool(name="mpool", bufs=8) as mp,
            tc.tile_pool(name="u2pool", bufs=2) as u2p,
            tc.tile_pool(name="stage", bufs=4) as sp,
            tc.tile_pool(name="bounce", bufs=1) as bp,
            tc.tile_pool(name="cpsum", bufs=2, space="PSUM") as cps,
            tc.tile_pool(name="upsum", bufs=2, space="PSUM") as ups,
            tc.tile_pool(name="ypsum", bufs=2, space="PSUM") as yps,
        ):
            # ---- pair gather table: tabL[r] = [flat[r] | flat[r+1]] ----
            # built via SBUF bounce (DRAM->DRAM DMA completion sems are
            # unreliable here); rows wrap as [128 partitions, 134 rows].
            tabl2 = tabl_d[:].rearrange("r (h c) -> r h c", h=2)
            for h in range(2):
                for cq in range(2):
                    r0 = cq * 8576
                    tb = bp.tile([128, 67 * 64], dt.bfloat16, tag="tb")
                    tbv = tb[:].rearrange("p (q c) -> p q c", c=64)
                    nc.sync.dma_start(
                        tbv,
                        xpt_v[h + r0:h + r0 + 8576].rearrange(
                            "(p q) c -> p q c", q=67))
                    nc.sync.dma_start(
                        tabl2[r0:r0 + 8576, h].rearrange(
                            "(p q) c -> p q c", q=67), tbv)

            # ---- constants / weights to SBUF ----
            fwl = pp.tile([128, 704], dt.bfloat16)
            nc.sync.dma_start(fwl[:], fwl_dv)
            fwv = fwl[:, 0:576].rearrange("p (nn o) -> p nn o", o=64)
            lt2 = fwl[0:2, 576:704]
            pw2 = pp.tile([64, 162], dt.bfloat16)
            nc.sync.dma_start(pw2[:], pw2_dv)
            pw2v = pw2[:].rearrange("c (uv m) -> c uv m", m=18)
            pnxt = pp.tile([128, 576], dt.float32)
            nc.sync.dma_start(pnxt[:], pnxt_c[:])
            byt = pp.tile([128, 576], dt.float32)
            nc.sync.dma_start(byt[:], byt_c[:])
            pm64 = pp.tile([128, 1], dt.float32)
            nc.sync.dma_start(pm64[:], pm64_c[:])
            bjs = pp.tile([128, 1], dt.float32)
            nc.sync.dma_start(bjs[:], bjs_c[:])

            # ---- rh = partition_id & 1, bxs = rh*64 + (p%64) + 1 ----
            pid_u = pp.tile([1, 1], dt.uint32)
            nc.sync.dma_start(pid_u[:], nc.partition_id_tensor[0:1, 0:1])
            pid0 = pp.tile([1, 1], dt.float32)
            nc.vector.tensor_copy(pid0[:], pid_u[:])
            pidf = pp.tile([128, 1], dt.float32)
            nc.gpsimd.partition_broadcast(pidf[:], pid0[:])
            t1 = pp.tile([128, 1], dt.float32)
            nc.vector.tensor_scalar(
                t1[:], pidf[:], 0.5, -0.25, op0=Alu.mult, op1=Alu.add)
            fl = pp.tile([128, 1], dt.float32)
            nc.vector.tensor_scalar(fl[:], t1[:], F23, F23, op0=Alu.add, op1=Alu.subtract)
            rh = pp.tile([128, 1], dt.float32)
            nc.vector.scalar_tensor_tensor(
                rh[:], fl[:], -2.0, pidf[:], op0=Alu.mult, op1=Alu.add)
            bxs = pp.tile([128, 1], dt.float32)
            nc.vector.scalar_tensor_tensor(
                bxs[:], rh[:], 64.0, pm64[:], op0=Alu.mult, op1=Alu.add)

            # ---- conv input: transpose to row-pair layout, lerp-select half ----
            xpair = pp.tile([128, 8512], dt.bfloat16)
            nc.sync.dma_start(xpair[:], xpair_v, transpose=True)
            diff = pp.tile([128, 4290], dt.bfloat16)
            nc.vector.tensor_tensor(
                diff[:], xpair[:, 4160:8450], xpair[:, 0:4290], op=Alu.subtract)
            xc = pp.tile([64, 8580], dt.bfloat16)
            xc3 = xc[:].rearrange("c (q par) -> c q par", par=2)
            nc.vector.scalar_tensor_tensor(
                xc3[:, :, 0], diff[0:64, :], rh[0:64, 0:1], xpair[0:64, 0:4290],
                op0=Alu.mult, op1=Alu.add)
            nc.vector.scalar_tensor_tensor(
                xc3[:, :, 1], diff[64:128, :], rh[64:128, 0:1],
                xpair[64:128, 0:4290], op0=Alu.mult, op1=Alu.add)

            # ---- phase 1: offset conv -> DRAM [18, 8192] (m on partitions) ----
            xcv = xc[:].rearrange("c (r w) -> c r w", w=Wp)
            offs_pv = offs_d[:].rearrange("(p m kk) -> p m kk", m=18, kk=KC)
            for ch in range(16):          # 16 chunks of 4 output rows (512 q)
                ps = cps.tile([18, 512], dt.float32, space="PSUM")
                i0 = ch * 4
                for uv in range(9):
                    u, v = uv // 3, uv % 3
                    rhs = xcv[:, i0 + u:i0 + u + 4, v:v + W]
                    nc.tensor.matmul(
                        ps[:], pw2v[:, uv], rhs,
                        start=(uv == 0), stop=(uv == 8),
                    )
                ost = sp.tile([18, 512], dt.float32, tag="ost")
                ps_v = ps[:].rearrange("m (i j) -> m i j", j=W)
                ost_v = ost[:].rearrange("m (jp i kk) -> m jp i kk", jp=2, kk=KC)
                for jp in range(2):
                    nc.scalar.copy(ost_v[:, jp], ps_v[:, :, jp::2])
                    nc.sync.dma_start(
                        offs_pv[jp * 64 + i0:jp * 64 + i0 + 4, :, :].rearrange(
                            "i m kk -> m i kk"),
                        ost_v[:, jp])
            offq = pp.tile([128, 18 * KC], dt.float32)
            nc.sync.dma_start(offq[:], offs_d[:].rearrange("(p c) -> p c", p=128))

            # ---- phase 2: weights/indices on [128, 576] tiles ----
            offx = offq[:, 0:576]
            offy = offq[:, 576:1152]

            def axis_weights(off, base_s, base_t, hi):
                p = wp1.tile([128, 576], dt.float32, tag="p")
                nc.vector.scalar_tensor_tensor(
                    p[:], off, base_s, base_t, op0=Alu.add, op1=Alu.add)
                f = wp1.tile([128, 576], dt.float32, tag="f")
                nc.vector.tensor_scalar(
                    f[:], p[:], F23 - 0.5, F23, op0=Alu.add, op1=Alu.subtract)
                q = wp.tile([128, 576], dt.float32, tag="q")
                nc.vector.tensor_scalar(
                    q[:], f[:], 0.0, float(hi - 1), op0=Alu.max, op1=Alu.min)
                pc = wp1.tile([128, 576], dt.float32, tag="pc")
                nc.vector.tensor_scalar(
                    pc[:], p[:], 0.0, float(hi), op0=Alu.max, op1=Alu.min)
                t = wp1.tile([128, 576], dt.float32, tag="t")
                nc.vector.tensor_tensor(t[:], pc[:], q[:], op=Alu.subtract)
                m0 = wp1.tile([128, 576], dt.float32, tag="m0")
                nc.vector.tensor_scalar(
                    m0[:], f[:], -0.5, 1.0, op0=Alu.is_le, op1=Alu.add)
                w0 = wp.tile([128, 576], dt.float32, tag="w0")
                nc.vector.tensor_tensor(w0[:], m0[:], t[:], op=Alu.subtract)
                m1 = wp1.tile([128, 576], dt.float32, tag="m1")
                nc.vector.tensor_scalar(m1[:], f[:], float(hi) - 0.5, None, op0=Alu.is_ge)
                w1 = wp.tile([128, 576], dt.float32, tag="w1")
                nc.vector.tensor_tensor(w1[:], t[:], m1[:], op=Alu.add)
                return q, w0, w1

            qx, a0, a1 = axis_weights(offx, bxs[:, 0:1], pnxt[:], Hp - 1)
            qy, w0, w1 = axis_weights(offy, bjs[:, 0:1], byt[:], Wp - 1)

            u_tiles = []
            for (wa, wb) in ((a0, w0), (a0, w1), (a1, w0), (a1, w1)):
                u = pp.tile([128, 576], dt.bfloat16, tag=f"u{len(u_tiles)}")
                nc.vector.tensor_tensor(u[:], wa[:], wb[:], op=Alu.mult)
                u_tiles.append(u)

            s_f = wp1.tile([128, 576], dt.float32, tag="sf")
            nc.vector.scalar_tensor_tensor(
                s_f[:], qx[:], 130.0, qy[:], op0=Alu.mult, op1=Alu.add)
            s16 = pp.tile([128, 576], dt.int16)
            nc.vector.tensor_copy(s16[:], s_f[:])

            # ---- phase 3: DRAM round-trips for idx + u rows ----
            # sidx_d layout: addr = P*4608 + nn*512 + r*32 + f, P = qpl*4+ks
            sidx_wv = sidx_d[:].rearrange(
                "(P nn r f) -> P nn r f", P=16, nn=9, r=NR, f=32)
            s16v = s16[:].rearrange("p (nn r ks) -> p nn r ks", nn=9, ks=4)
            for qpl in range(4):
                for ks in range(4):
                    nc.sync.dma_start(
                        sidx_wv[qpl * 4 + ks].rearrange("nn r f -> f nn r"),
                        s16v[qpl::4, :, :, ks])
            idx = pp.tile([128, 9 * NR * 32], dt.int16)
            idxv = idx[:].rearrange("p (nn r f) -> p nn r f", nn=9, r=NR)
            nc.sync.dma_start(idx[0:16, :], sidx_d[:].rearrange("(P c) -> P c", P=16))
            for g in range(1, 8):
                nc.sync.dma_start(idx[g * 16:(g + 1) * 16, :], idx[0:16, :])
            idxB = pp.tile([128, 9 * NR * 32], dt.int16)
            nc.vector.tensor_scalar(idxB[:], idx[:], 130, None, op0=Alu.add)
            idxBv = idxB[:].rearrange("p (nn r f) -> p nn r f", nn=9, r=NR)

            # u4_d layout: addr = cn*73728 + r*4608 + nn*512 + qp*4 + ks
            u4_wv = u4_d[:].rearrange(
                "(cn r nn qp ks) -> cn r nn qp ks", cn=4, r=NR, nn=9, ks=4)
            u4_pv = u4_d[:].rearrange(
                "(cn2 cnl r c) -> cn2 cnl r c", cn2=2, cnl=2, r=NR)
            for ci, u in enumerate(u_tiles):
                uv3 = u[:].rearrange("p (nn r ks) -> p nn r ks", nn=9, ks=4)
                for r2 in range(NR):
                    nc.sync.dma_start(
                        u4_wv[ci, r2].rearrange("nn qp ks -> qp nn ks"),
                        uv3[:, :, r2, :])

            # ---- phase 4: gather + weight + fold ----
            tabv = tabl_d[:]
            nreg = nc.gpsimd.to_reg(TW)
            for r2 in range(NR):
                u2tb = u2p.tile([2, 2 * 9 * TW], dt.bfloat16, tag="u2tb")
                nc.sync.dma_start(
                    u2tb[:].rearrange("p (cn2 c) -> p cn2 c", cn2=2),
                    u4_pv[:, :, r2].rearrange("cn2 cnl c -> cnl cn2 c"))
                y = yps.tile([64, TW], dt.float32, space="PSUM")
                for n in range(9):
                    k = r2 * 9 + n
                    g = gp.tile([128, 2, TW], dt.bfloat16, tag="g")
                    nc.gpsimd.dma_gather(
                        g[:, 0:1, :], tabv, idxv[:, n, r2, :], TW, nreg, 128,
                        transpose=True, queue_num=0,
                    )
                    nc.gpsimd.dma_gather(
                        g[:, 1:2, :], tabv, idxBv[:, n, r2, :], TW, nreg, 128,
                        transpose=True, queue_num=0,
                    )
                    utb = ups.tile([128, 2, TW], dt.float32, space="PSUM", tag="utb")
                    nc.tensor.matmul(
                        utb[:, 0, :], lt2[:], u2tb[:, n * TW:(n + 1) * TW],
                        start=True, stop=True)
                    nc.tensor.matmul(
                        utb[:, 1, :], lt2[:],
                        u2tb[:, 9 * TW + n * TW:9 * TW + (n + 1) * TW],
                        start=True, stop=True)
                    m2 = mp.tile([128, 2, TW], dt.bfloat16, tag="m2")
                    nc.vector.tensor_tensor(m2[:], g[:], utb[:], op=Alu.mult)
                    nc.tensor.matmul(
                        y[:], fwv[:, n], m2[:, 0, :], start=(n == 0), stop=False)
                    nc.tensor.matmul(
                        y[:], fwv[:, n], m2[:, 1, :], start=False, stop=(n == 8))
                st = sp.tile([64, TW], dt.bfloat16, tag="st")
                nc.vector.tensor_copy(
                    st[:].rearrange("o (i ks jp) -> o i ks jp", i=HALF, ks=4),
                    y[:].rearrange("o (jp i ks) -> o i ks jp", jp=2, i=HALF))
                nc.sync.dma_start(
                    out_d[:, :, 8 * r2:8 * r2 + 8],
                    st[:].rearrange("o (i j) -> o i j", j=8))

    nc.compile()
    return nc


def _make_runner():
    import jax
    import jax.numpy as jnp
    from jax.sharding import Mesh, PartitionSpec, NamedSharding
    try:
        from jax.experimental.shard_map import shard_map
    except ImportError:
        from jax.sharding import shard_map  # newer jax
    from concourse import bass2jax
    import concourse.mybir as mybir

    # The libneuronxla NEFF cache is keyed on an outer-HLO hash that does not
    # cover the embedded BIR, so a changed kernel can silently reuse a stale
    # NEFF.  Clear it so this build's program is what actually runs.
    import shutil
    shutil.rmtree(os.path.expanduser("~/.neuron-compile-cache"), ignore_errors=True)

    nc = _build_program()
    bass2jax.install_neuronx_cc_hook()
    assert not getattr(nc, "dbg_callbacks", None)

    partition_name = (
        nc.partition_id_tensor.name if nc.partition_id_tensor is not None else None)
    in_names, out_names, out_avals, zero_specs = [], [], [], []
    for alloc in nc.m.functions[0].allocations:
        if not isinstance(alloc, mybir.MemoryLocationSet):
            continue
        name = alloc.memorylocations[0].name
        if alloc.kind == "ExternalInput":
            if name != partition_name:
                in_names.append(name)
        elif alloc.kind == "ExternalOutput":
            out_names.append(name)
            shape = tuple(alloc.tensor_shape)
            dtype = mybir.dt.np(alloc.dtype)
            out_avals.append(jax.core.ShapedArray(shape, dtype))
            zero_specs.append((shape, dtype))
    n_params = len(in_names)
    n_outs = len(out_names)
    in_names_all = tuple(in_names) + tuple(out_names)
    if partition_name is not None:
        in_names_all = in_names_all + (partition_name,)
    donate = tuple(range(n_params, n_params + n_outs))

    def _body(*args):
        operands = list(args)
        if partition_name is not None:
            operands.append(bass2jax.partition_id_tensor())
        outs = bass2jax._bass_exec_p.bind(
            *operands,
            out_avals=tuple(out_avals),
            in_names=in_names_all,
            out_names=tuple(out_names),
            lowering_input_output_aliases=(),
            sim_require_finite=True,
            sim_require_nnan=True,
            nc=nc,
        )
        return tuple(outs)

    devices = jax.devices()[:8]
    mesh = Mesh(np.asarray(devices), ("core",))
    in_specs = (PartitionSpec("core"),) * (n_params + n_outs)
    out_specs = (PartitionSpec("core"),) * n_outs
    sharded = jax.jit(
        shard_map(_body, mesh=mesh, in_specs=in_specs, out_specs=out_specs,
                  check_rep=False),
        donate_argnums=donate, keep_unused=True)
    zsh = NamedSharding(mesh, PartitionSpec("core"))
    mkzeros = jax.jit(
        lambda: tuple(jnp.zeros((8 * s[0], *s[1:]), d) for s, d in zero_specs),
        out_shardings=(zsh,) * n_outs)
    return {
        "sharded": sharded, "mkzeros": mkzeros,
        "in_names": in_names, "out_names": out_names,
    }


def _prep_blob(x, p_dw, p_pw, c_dw, c_pw):
    big = np.zeros((8, TOTAL), BF16)
    xt = np.ascontiguousarray(x.transpose(0, 2, 3, 1)).astype(BF16)  # [b,h,w,c]
    for b in range(4):
        xv = big[2 * b, OFF_XPT:OFF_XPT + XPT_E].reshape(NXPT, 64)
        img = xv[:Hp * Wp].reshape(Hp, Wp, 64)
        img[1:129, 1:129, :] = xt[b]
        big[2 * b + 1, OFF_XPT:OFF_XPT + XPT_E] = big[2 * b, OFF_XPT:OFF_XPT + XPT_E]
    p = np.arange(128)
    pw2 = (p_pw[:, :, 0, 0].T[:, None, :]
           * p_dw[:, 0].reshape(C, 9)[:, :, None]).astype(BF16)   # [c, uv, m]
    fwp = (c_dw[p % 64, 0].reshape(128, 9)[:, :, None]
           * c_pw[:, p % 64, 0, 0].T[:, None, :]).astype(BF16)    # [p, n, o]
    fwl = np.zeros((128, 704), BF16)
    fwl[:, 0:576] = fwp.reshape(128, 576)
    fwl[0, 576:640] = 1.0
    fwl[1, 640:704] = 1.0
    big[:, OFF_FWL:OFF_FWL + FWL_E] = fwl.reshape(-1)[None, :]
    big[:, OFF_PW2:OFF_PW2 + PW2_E] = pw2.reshape(-1)[None, :]
    return big


def kernel(x, p_dw, p_pw, c_dw, c_pw):
    x = np.asarray(x, np.float32)
    p_dw = np.asarray(p_dw, np.float32)
    p_pw = np.asarray(p_pw, np.float32)
    c_dw = np.asarray(c_dw, np.float32)
    c_pw = np.asarray(c_pw, np.float32)

    if "runner" not in _cache:
        _cache["runner"] = _make_runner()
    r = _cache["runner"]

    big = _prep_blob(x, p_dw, p_pw, c_dw, c_pw)
    args = []
    for name in r["in_names"]:
        if name == "blob":
            args.append(big.reshape(-1))
        elif name == "dbg_addr":
            args.append(np.zeros((8, 2), np.uint32))
        else:
            raise RuntimeError(f"unexpected input {name}")
    zeros = _cache.pop("zeros", None) or r["mkzeros"]()
    outs = r["sharded"](*args, *zeros)
    _cache["zeros"] = r["mkzeros"]()  # pre-dispatch for the next call
    oi = r["out_names"].index("out")
    res = np.asarray(outs[oi]).astype(np.float32).reshape(8, C, HALF, W)

    out = np.empty((B, C, H, W), np.float32)
    for core in range(8):
        b, rh = core // 2, core % 2
        out[b, :, rh * 64:(rh + 1) * 64, :] = res[core]
    return out


if __name__ == "__main__":
    import npref
    inp = npref.get_inputs()
    got = kernel(**inp)
    exp = np.load("/tmp/ref_out.npy")
    err = np.abs(got - exp).max()
    print("absmax err:", err, "rel:", err / np.abs(exp).max())
